# revision 32
# baseline (speedup 1.0000x reference)
"""BertSelfAttention (synthesizer mixture + symmetric ALiBi) Bass kernel for TRN2.

Data-parallel over batch: 8 cores x 2 batches each. One SPMD program.

Decomposition (per core, batches b=0,1; heads h=0..11):
  mw = softmax(mixture_weight)                          (host, 24 floats)
  aexp[h,j,i] = exp(mw1_h*synth_h[i,j] - slope_h*|i-j|) (host table, band-packed)
  projections: fp8 "comp3" — x = xh + xl, 64*W = Wh + Wl (all e4m3);
    acc = Wh.xh + Wh.xl + Wl.xh   (9 DoubleRow matmuls per 128x512 out tile,
    2 contraction k-tiles per matmul, 0.5 cyc/row => 3x the fp32r rate)
  qT holds 64*mw0/sqrt(64)*q, kT holds 64*k  (bf16 in SBUF)
  scT[j,i] = kT_h^T qT_h = 4096 * sc_true               (PE, bf16)
  eT = exp(scT * 1/4096) ; eT *= aexp[h]                (ACT exp w/ scale; DVE/Pool mul)
  v built with a ones-column per head (65-wide heads; v scaled back by 1/64
  on the ACT-engine psum evacuation);
  [ctx | rowsum] = eT^T @ [v | 1]                       (PE, one chain per it,
    4 it-chains batched in one PSUM tile per (h,b))
  out[i, PERM[h]*64:] = ctx * (1/rowsum)                (DVE recip + DVE bcast mul)
  out staged bf16 in true-head column order; stored in 4-head (512B) chunks.

ALiBi banding: (jt,it) 128x128 tile pairs with exp(-slope*dist) below ~3e-5
relative contribution are skipped (scores, exp, mul, pv, and table storage:
the aexp table is host-packed to only the kept spans).
"""

from contextlib import ExitStack

import numpy as np

import concourse.bass as bass
import concourse.mybir as mybir
import concourse.tile as tile

F32 = mybir.dt.float32
BF16 = mybir.dt.bfloat16
F8 = mybir.dt.float8e4

H, S, D, DH = 12, 512, 768, 64
BPC = 2                # batches per core
T = BPC * S            # tokens per core
KT = D // 128          # contraction tiles over model dim
MT = T // 128          # token tiles per core
JT = S // 128          # key tiles per sequence
VW = DH + 1            # per-head v width incl. ones column
BAND_MARGIN = 10.5

# combined fp8 input layout: columns of xw8 [D, XW_COLS]
#   X region:  xh | xl                      (2T cols)
#   QK region: per gi: wq_h | wq_l | wk_h | wk_l   (6 x 512 cols)
#   V region:  wv_h | wv_l                  (2D cols)
XW_XH = 0
XW_XL = T
XW_QK = 2 * T
XW_V = XW_QK + KT * 512
XW_COLS = XW_V + 2 * D


def _get_slopes(n):
    import math

    def pow2(n):
        start = 2 ** (-(2 ** (-(math.log2(n) - 3))))
        return [start * start**i for i in range(n)]

    if math.log2(n).is_integer():
        return pow2(n)
    cp2 = 2 ** math.floor(math.log2(n))
    return pow2(cp2) + _get_slopes(2 * cp2)[0::2][: n - cp2]


SLOPES = np.asarray(_get_slopes(H), np.float64)


def _band_dt(band_margin: float) -> list[int]:
    """Max |jt-it| (inclusive) per head; JT-1 means no banding."""
    out = []
    for sl in SLOPES:
        L = int(np.ceil(band_margin / sl))
        out.append(min((L + 127) // 128, JT - 1))
    return out


BAND = _band_dt(BAND_MARGIN)

# slot -> original head. Pairs (2gi, 2gi+1) mix a wide-band (costly exp) head
# with a narrow one so the Activation engine's load is even across the
# kernel; true-head quads (0-3, 4-7, 8-11) complete early so output stores
# can go out in contiguous 512B chunks.
PERM = [0, 4, 11, 5, 1, 6, 2, 7, 3, 10, 8, 9]
# true-head quad q covers out cols [256q, 256q+256); SLOT_OF[h] = slot of h
SLOT_OF = [PERM.index(h) for h in range(H)]
QUAD_DONE = [max(SLOT_OF[4 * q : 4 * q + 4]) for q in range(3)]  # slot finishing quad


def _spans(s):
    """Per jt: (i0, iw) kept tile-rounded column range for head slot s."""
    dt_h = BAND[PERM[s]]
    out = []
    for jt in range(JT):
        lo = max(0, jt - dt_h)
        hi = min(JT - 1, jt + dt_h)
        out.append((lo * 128, (hi - lo + 1) * 128))
    return out


def _exact_spans(s):
    """Per jt: (e0, e1) columns where exp(-slope*dist) is non-negligible.
    Score matmuls only write these; the rest of the tile-rounded span is
    memset in eT and never multiplied, so the table only stores the exact
    spans."""
    L = int(np.ceil(BAND_MARGIN / SLOPES[PERM[s]]))
    out = []
    for jt, (i0, iw) in enumerate(_spans(s)):
        e0 = max(i0, jt * 128 - L)
        e1 = min(i0 + iw, (jt + 1) * 128 + L)
        out.append((e0, e1))
    return out


# packed aexp column offsets: per (slot, jt) block of exact-span width
AE_OFF = {}
_c = 0
for _s in range(H):
    for _jt, (_e0, _e1) in enumerate(_exact_spans(_s)):
        AE_OFF[(_s, _jt)] = _c
        _c += _e1 - _e0
AE_COLS = _c


def _patch_tile_drain():
    """This walrus build rejects >1 sync-wait on one instruction; split the
    TileContext tail-drain's waits across single-wait drains."""
    from concourse.vector_clock import ScopedClock

    def _drain_and_barrier(self, tick_clock, wait_clock):
        nc = self.nc
        drain_inst = nc.sync.drain()
        wait_clock.add_sem_waits(
            drain_inst.ins, ScopedClock({None: tick_clock.global_clock})
        )
        waits = list(drain_inst.ins.sync_info.on_wait)
        if len(waits) > 1:
            drain_inst.ins.sync_info.on_wait = waits[:1]
            for w in waits[1:]:
                extra = nc.sync.drain()
                extra.ins.sync_info = mybir.SyncInfo(on_wait=[w], on_update=[])
        nc.all_engine_barrier()
        assert self.sems is not None
        popped = nc._tile_sem_poison_stack.pop()
        assert popped is self._sem_poison
        nc.clear_and_free_semaphores(list(self.sems.allocated().values()))
        nc.all_engine_barrier()

    tile.TileContext._drain_and_barrier = _drain_and_barrier


_patch_tile_drain()


def _split_multi_waits(nc):
    """This walrus build accepts at most one sync-wait per instruction; hoist
    extra waits onto single-wait NOPs emitted just before, on the same engine."""
    for fn in nc.m.functions:
        for bb in fn.blocks:
            out = []
            changed = False
            for ins in bb.instructions:
                si = ins.sync_info
                if si is not None and si.on_wait and len(si.on_wait) > 1:
                    waits = list(si.on_wait)
                    for i, w in enumerate(waits[:-1]):
                        nop = mybir.InstNoOp(
                            name=f"{ins.name}_w{i}",
                            engine=ins.engine,
                            sync_info=mybir.SyncInfo(on_wait=[w], on_update=[]),
                            bass_nofuse=True,
                        )
                        nc.register_instruction(nop, overwrite=True)
                        out.append(nop)
                    si.on_wait = waits[-1:]
                    changed = True
                out.append(ins)
            if changed:
                bb.instructions = out


def build_nc() -> bass.Bass:
    nc = bass.Bass("TRN2")
    xw8 = nc.dram_tensor("xw8", [D, XW_COLS], F8, kind="ExternalInput").ap()
    aexpP = nc.dram_tensor("aexpP", [128, AE_COLS], BF16, kind="ExternalInput").ap()
    out = nc.dram_tensor("out", [T, D], BF16, kind="ExternalOutput").ap()
    outR = out.rearrange("(mt p) d -> p mt d", p=128)

    DRM = mybir.MatmulPerfMode.DoubleRow

    with tile.TileContext(nc) as tc, ExitStack() as ctx:
        pers = ctx.enter_context(tc.tile_pool(name="pers", bufs=1))
        exp_p = ctx.enter_context(tc.tile_pool(name="exp_p", bufs=22))
        r_p = ctx.enter_context(tc.tile_pool(name="r_p", bufs=4))
        psA = ctx.enter_context(tc.tile_pool(name="psA", bufs=2, space="PSUM"))
        psS = ctx.enter_context(tc.tile_pool(name="psS", bufs=4, space="PSUM"))
        psC = ctx.enter_context(tc.tile_pool(name="psC", bufs=2, space="PSUM"))

        xw_sb = pers.tile([128, KT, XW_COLS], F8, tag="xw")
        ae_sb = pers.tile([128, AE_COLS], BF16, tag="ae")
        qT_sb = pers.tile([128, KT, T], BF16, tag="qT")
        kT_sb = pers.tile([128, KT, T], BF16, tag="kT")
        v_sb = pers.tile([128, MT, H * VW], BF16, tag="v")
        out_sb = pers.tile([128, MT, D], BF16, tag="outsb")

        def xh(g, cols):
            return xw_sb[:, 2 * g : 2 * g + 2, XW_XH + cols[0] : XW_XH + cols[1]]

        def xl(g, cols):
            return xw_sb[:, 2 * g : 2 * g + 2, XW_XL + cols[0] : XW_XL + cols[1]]

        def wqk(wi, hi_lo, gi, g):
            c0 = XW_QK + gi * 512 + (2 * wi + hi_lo) * 128
            return xw_sb[:, 2 * g : 2 * g + 2, c0 : c0 + 128]

        def wv(hi_lo, g, cols):
            c0 = XW_V + hi_lo * D
            return xw_sb[:, 2 * g : 2 * g + 2, c0 + cols[0] : c0 + cols[1]]

        # ones columns of v: cx[:, DH] accumulates the softmax denominator
        ones_cols = v_sb.rearrange("p m (h w) -> p m h w", w=VW)[:, :, :, DH : DH + 1]
        nc.gpsimd.memset(ones_cols, 1.0)

        # ---- input DMAs, ordered by first use ----
        xw8R = xw8.rearrange("(kt p) c -> p kt c", p=128)

        def ae_load(s, n=1):
            c0 = AE_OFF[(s, 0)]
            c1 = AE_OFF[(s + n, 0)] if s + n < H else AE_COLS
            nc.sync.dma_start(out=ae_sb[:, c0:c1], in_=aexpP[:, c0:c1])

        def qk_load(gi, k0=0, k1=KT):
            c0 = XW_QK + gi * 512
            nc.sync.dma_start(
                out=xw_sb[:, k0:k1, c0 : c0 + 512],
                in_=xw8R[:, k0:k1, c0 : c0 + 512],
            )

        def x_load(kt):
            nc.sync.dma_start(
                out=xw_sb[:, kt, 0 : 2 * T],
                in_=xw8[kt * 128 : (kt + 1) * 128, 0 : 2 * T],
            )

        # stagger x and the first w-column loads so the first projection
        # chain's g=0 matmuls can start ~4us in instead of waiting for all
        x_load(0)
        x_load(1)
        qk_load(0, 0, 2)
        x_load(2)
        x_load(3)
        qk_load(0, 2, 4)
        x_load(4)
        x_load(5)
        qk_load(0, 4, 6)
        ae_load(0)
        ae_load(1)
        for hi_lo in (0, 1):
            c0 = XW_V + hi_lo * D
            nc.sync.dma_start(
                out=xw_sb[:, :, c0 : c0 + D], in_=xw8R[:, :, c0 : c0 + D]
            )
        qk_load(1)
        qk_load(2)
        ae_load(2)
        qk_load(3)
        ae_load(3)
        qk_load(4)
        ae_load(4)
        qk_load(5)
        ae_load(5)
        ae_load(6)
        ae_load(7)
        ae_load(8, 4)

        TERMS = ((0, xh), (0, xl), (1, xh))  # Wh.xh + Wh.xl + Wl.xh

        def proj_qk(gi):
            for wi, dst in ((0, qT_sb), (1, kT_sb)):
                for nt in range(T // 512):
                    ps = psA.tile([128, 512], F32, tag="psA", name=f"psA_{wi}{gi}{nt}")
                    n = 0
                    for g in range(KT // 2):
                        for hi_lo, xf in TERMS:
                            nc.tensor.matmul(
                                ps,
                                lhsT=wqk(wi, hi_lo, gi, g),
                                rhs=xf(g, (nt * 512, (nt + 1) * 512)),
                                start=(n == 0),
                                stop=(n == 8),
                                perf_mode=DRM,
                            )
                            n += 1
                    nc.vector.tensor_copy(
                        out=dst[:, gi, nt * 512 : (nt + 1) * 512], in_=ps
                    )

        def proj_v(mt):
            for half, (n0, nw, h0, nh) in enumerate(
                ((0, 512, 0, 8), (512, 256, 8, 4))
            ):
                ps = psA.tile([128, 512], F32, tag="psA", name=f"psA_v{mt}{half}")
                n = 0
                for g in range(KT // 2):
                    for hi_lo, xf in TERMS:
                        nc.tensor.matmul(
                            ps[:, :nw],
                            lhsT=xf(g, (mt * 128, (mt + 1) * 128)),
                            rhs=wv(hi_lo, g, (n0, n0 + nw)),
                            start=(n == 0),
                            stop=(n == 8),
                            perf_mode=DRM,
                        )
                        n += 1
                dst = v_sb[:, mt, h0 * VW : (h0 + nh) * VW]
                dst = dst.rearrange("p (h w) -> p h w", w=VW)[:, :, 0:DH]
                src = ps[:, :nw].rearrange("p (h w) -> p h w", w=DH)
                nc.vector.tensor_scalar_mul(out=dst, in0=src, scalar1=1.0 / 64.0)

        eTs = {}

        def attend_scores(s):
            po, gi = (s % 2) * 64, s // 2
            spans = _spans(s)
            exact = _exact_spans(s)
            full = BAND[PERM[s]] == JT - 1
            for pair in (0, 1):
                jts = (2 * pair, 2 * pair + 1)
                for b in range(BPC):
                    t0 = b * S
                    eT = exp_p.tile(
                        [128, 1024], BF16, tag="eT", name=f"eT{s}_{b}_{pair}"
                    )
                    # zero the in-band-but-negligible strips PV will read;
                    # exp/mul only touch the exact spans
                    for k, jt in enumerate(jts):
                        i0, iw = spans[jt]
                        e0, e1 = exact[jt]
                        if e0 > i0:
                            nc.gpsimd.memset(
                                eT[:, k * 512 : k * 512 + e0 - i0], 0.0
                            )
                        if e1 < i0 + iw:
                            nc.gpsimd.memset(
                                eT[:, k * 512 + e1 - i0 : k * 512 + iw], 0.0
                            )
                    # one [128,512] psum tile per jt: a single matmul writer
                    # per tile keeps Tile's cross-engine RAW deps sound while
                    # letting both matmuls run before the exps (pipelined)
                    scs = []
                    for k, jt in enumerate(jts):
                        i0, iw = spans[jt]
                        e0, e1 = exact[jt]
                        sck = psS.tile(
                            [128, 512], F32, tag="sc", name=f"sc{s}_{b}_{pair}_{k}"
                        )
                        nc.tensor.matmul(
                            sck[:, e0 - i0 : e1 - i0],
                            lhsT=kT_sb[
                                po : po + DH, gi, t0 + jt * 128 : t0 + (jt + 1) * 128
                            ],
                            rhs=qT_sb[po : po + DH, gi, t0 + e0 : t0 + e1],
                            start=True,
                            stop=True,
                        )
                        scs.append(sck)
                    if s >= 8 and b == 1:
                        # tail slots' lagged-batch muls go to the Pool
                        # engine (idle late); Pool mis-executes in-place RMW,
                        # so exp lands in a scratch tile and the mul writes
                        # eT fresh
                        ex = exp_p.tile(
                            [128, 1024], BF16, tag="eT", name=f"ex{s}_{b}_{pair}"
                        )
                        for k, jt in enumerate(jts):
                            i0, iw = spans[jt]
                            e0, e1 = exact[jt]
                            nc.scalar.activation(
                                out=ex[:, k * 512 + e0 - i0 : k * 512 + e1 - i0],
                                in_=scs[k][:, e0 - i0 : e1 - i0],
                                func=mybir.ActivationFunctionType.Exp,
                                scale=1.0 / 4096.0,
                            )
                            c0 = AE_OFF[(s, jt)]
                            nc.gpsimd.tensor_mul(
                                out=eT[:, k * 512 + e0 - i0 : k * 512 + e1 - i0],
                                in0=ex[:, k * 512 + e0 - i0 : k * 512 + e1 - i0],
                                in1=ae_sb[:, c0 : c0 + e1 - e0],
                            )
                    else:
                        for k, jt in enumerate(jts):
                            i0, iw = spans[jt]
                            e0, e1 = exact[jt]
                            nc.scalar.activation(
                                out=eT[:, k * 512 + e0 - i0 : k * 512 + e1 - i0],
                                in_=scs[k][:, e0 - i0 : e1 - i0],
                                func=mybir.ActivationFunctionType.Exp,
                                scale=1.0 / 4096.0,
                            )
                            c0 = AE_OFF[(s, jt)]
                            nc.vector.tensor_mul(
                                out=eT[:, k * 512 + e0 - i0 : k * 512 + e1 - i0],
                                in0=eT[:, k * 512 + e0 - i0 : k * 512 + e1 - i0],
                                in1=ae_sb[:, c0 : c0 + e1 - e0],
                            )
                    eTs[(s, pair, b)] = eT

        def attend_pv(s, b):
            spans = _spans(s)
            dt_h = BAND[PERM[s]]
            cx = psC.tile([128, JT * VW], F32, tag="cx", name=f"cx{s}_{b}")
            for it in range(JT):
                jts = [jt for jt in range(JT) if abs(jt - it) <= dt_h]
                for n, jt in enumerate(jts):
                    pair, k = divmod(jt, 2)
                    i0 = spans[jt][0]
                    off = k * 512 + (it * 128 - i0)
                    nc.tensor.matmul(
                        cx[:, it * VW : (it + 1) * VW],
                        lhsT=eTs[(s, pair, b)][:, off : off + 128],
                        rhs=v_sb[:, b * JT + jt, s * VW : (s + 1) * VW],
                        start=(n == 0),
                        stop=(n == len(jts) - 1),
                    )
            cx4 = cx.rearrange("p (i w) -> p i w", w=VW)
            r = r_p.tile([128, JT, 1], F32, tag="r", name=f"r{s}_{b}")
            nc.vector.reciprocal(out=r, in_=cx4[:, :, DH : DH + 1])
            c0 = PERM[s] * DH
            nc.vector.tensor_mul(
                out=out_sb[:, b * JT : (b + 1) * JT, c0 : c0 + DH],
                in0=cx4[:, :, 0:DH],
                in1=r.broadcast_to([128, JT, DH]),
            )

        def store_q(q, b, half=None):
            c0 = q * 4 * DH
            w = 4 * DH
            if half is not None:
                c0 += half * 2 * DH
                w = 2 * DH
            m0, m1 = b * JT, (b + 1) * JT
            nc.sync.dma_start(
                out=outR[:, m0:m1, c0 : c0 + w],
                in_=out_sb[:, m0:m1, c0 : c0 + w],
            )

        # warm up the PE p-state during the input-DMA window: matmuls on a
        # locally-memset tile keep the clock ramping toward 2.4 GHz so the
        # real projection chains run at full speed
        warm = pers.tile([128, 256], BF16, tag="warm")
        nc.vector.memset(warm, 0.0)
        wps = psS.tile([128, 512], F32, tag="sc", name="warmps")
        NWARM = 24
        for i in range(NWARM):
            nc.tensor.matmul(
                wps[:, 0:256],
                lhsT=warm[:, 0:128],
                rhs=warm,
                start=(i == 0),
                stop=(i == NWARM - 1),
            )

        # schedule: projections pipelined one block ahead (proj(gi+1) emitted
        # mid-block as PE filler while exp(s0)/mul latency drains), v and PV
        # staggered per batch so PE always has work while ACT catches up.
        proj_qk(0)
        attend_scores(0)
        attend_scores(1)
        for mt in range(4):
            proj_v(mt)
        attend_pv(0, 0)
        attend_pv(1, 0)
        proj_qk(1)
        attend_scores(2)
        attend_scores(3)
        for mt in range(4, 8):
            proj_v(mt)
        attend_pv(0, 1)
        attend_pv(1, 1)
        proj_qk(2)
        attend_pv(2, 0)
        attend_pv(3, 0)
        # blocks 2..5: lagged b=1 PVs fill PE while exp of fresh scores
        # drains; the four narrow tail heads' scores are pulled forward so
        # the final stretch is pure pre-buffered PV work
        attend_scores(4)
        attend_scores(5)
        attend_pv(2, 1)
        attend_pv(3, 1)
        proj_qk(3)
        attend_pv(4, 0)
        attend_pv(5, 0)
        attend_scores(6)
        attend_scores(7)
        attend_pv(4, 1)
        attend_pv(5, 1)
        proj_qk(4)
        attend_pv(6, 0)
        attend_pv(7, 0)
        store_q(1, 0)
        proj_qk(5)
        attend_scores(8)
        attend_scores(9)
        attend_pv(6, 1)
        attend_pv(7, 1)
        store_q(1, 1)
        attend_scores(10)
        attend_scores(11)
        attend_pv(8, 0)
        store_q(0, 0)
        attend_pv(9, 0)
        attend_pv(8, 1)
        store_q(0, 1)
        attend_pv(9, 1)
        attend_pv(10, 0)
        attend_pv(11, 0)
        store_q(2, 0)
        store_q(2, 1, half=1)
        attend_pv(10, 1)
        attend_pv(11, 1)
        store_q(2, 1, half=0)
    _split_multi_waits(nc)
    return nc


def host_prep(inputs: dict):
    """Returns (shared inputs dict, per-core xw8 list)."""
    import ml_dtypes

    E4 = ml_dtypes.float8_e4m3

    hs = np.ascontiguousarray(np.asarray(inputs["hidden_states"], np.float32))
    Wq = np.asarray(inputs["Wq"], np.float32)
    Wk = np.asarray(inputs["Wk"], np.float32)
    Wv = np.asarray(inputs["Wv"], np.float32)
    qfc = np.asarray(inputs["query_fc"], np.float32)
    kfc = np.asarray(inputs["key_fc"], np.float32)
    mwt = np.asarray(inputs["mixture_weight"], np.float32)[0, :, 0, 0, :]  # [H,2]

    e = np.exp(mwt - mwt.max(-1, keepdims=True))
    mw = e / e.sum(-1, keepdims=True)
    scale = np.repeat(mw[:, 0] / np.sqrt(DH), DH).astype(np.float32)

    def permute_heads(wT):  # [D_in, D_out]: reorder out-columns to slot order
        blocks = [wT[:, PERM[s] * DH : (PERM[s] + 1) * DH] for s in range(H)]
        return np.concatenate(blocks, axis=1)

    def hilo(wT):  # [D_in, D_out] -> fp8 hi, lo of 64*wT in slot order
        w64 = permute_heads(np.asarray(wT, np.float32)) * 64.0
        hi = w64.astype(E4)
        lo = (w64 - hi.astype(np.float32)).astype(E4)
        return hi, lo

    wq_h, wq_l = hilo((Wq * scale[:, None]).T)
    wk_h, wk_l = hilo(Wk.T)
    wv_h, wv_l = hilo(Wv.T)

    # packed band-restricted bias table [128, AE_COLS]
    synthT = np.einsum("hik,hjk->hji", qfc, kfc).astype(np.float32)
    pos = np.arange(S)
    absd = np.abs(pos[None, :] - pos[:, None]).astype(np.float32)
    slopes = SLOPES.astype(np.float32)
    bias = mw[:, 1][:, None, None] * synthT - slopes[:, None, None] * absd[None]
    aexp = np.exp(bias)  # [h, j, i]
    aeP = np.zeros((128, AE_COLS), np.float32)
    for s in range(H):
        for jt, (e0, e1) in enumerate(_exact_spans(s)):
            c = AE_OFF[(s, jt)]
            aeP[:, c : c + e1 - e0] = aexp[
                PERM[s], jt * 128 : (jt + 1) * 128, e0:e1
            ]
    aeP = np.ascontiguousarray(aeP.astype(ml_dtypes.bfloat16))

    shared = dict(aexpP=aeP)
    n_cores = hs.shape[0] // BPC
    xw8s = []
    for c in range(n_cores):
        xT = hs[c * BPC : (c + 1) * BPC].reshape(T, D).T  # [D, T]
        x_h = xT.astype(E4)
        x_l = (xT - x_h.astype(np.float32)).astype(E4)
        xw = np.empty((D, XW_COLS), E4)
        xw[:, XW_XH : XW_XH + T] = x_h
        xw[:, XW_XL : XW_XL + T] = x_l
        for gi in range(KT):
            c0 = XW_QK + gi * 512
            gc = slice(gi * 128, (gi + 1) * 128)
            xw[:, c0 : c0 + 128] = wq_h[:, gc]
            xw[:, c0 + 128 : c0 + 256] = wq_l[:, gc]
            xw[:, c0 + 256 : c0 + 384] = wk_h[:, gc]
            xw[:, c0 + 384 : c0 + 512] = wk_l[:, gc]
        xw[:, XW_V : XW_V + D] = wv_h
        xw[:, XW_V + D : XW_V + 2 * D] = wv_l
        xw8s.append(np.ascontiguousarray(xw))
    return shared, xw8s


# ---------------------------------------------------------------------------
# Harness entry point: full (unsharded) inputs -> full output.
# Shards batch 16 -> 8 cores x 2, runs the SPMD Bass kernel, gathers.
# ---------------------------------------------------------------------------

N_CORES = 8
_NC_CACHE: dict = {}


def kernel(**inputs) -> np.ndarray:
    shared, xw8s = host_prep(inputs)
    if "nc" not in _NC_CACHE:
        _NC_CACHE["nc"] = build_nc()
    nc = _NC_CACHE["nc"]
    in_maps = [dict(shared, xw8=xw8s[c]) for c in range(N_CORES)]
    from concourse.bass_utils import run_bass_kernel_spmd

    res = run_bass_kernel_spmd(nc, in_maps, core_ids=list(range(N_CORES)))
    outs = [
        np.asarray(res.results[c]["out"]).astype(np.float32).reshape(BPC, S, D)
        for c in range(N_CORES)
    ]
    return np.concatenate(outs, axis=0)


# revision 33
# speedup vs baseline: 1.0366x; 1.0366x over previous
"""BertSelfAttention (synthesizer mixture + symmetric ALiBi) Bass kernel for TRN2.

Data-parallel over batch: 8 cores x 2 batches each. One SPMD program.

Decomposition (per core, batches b=0,1; heads h=0..11):
  mw = softmax(mixture_weight)                          (host, 24 floats)
  aexp[h,j,i] = exp(mw1_h*synth_h[i,j] - slope_h*|i-j|) (host table, band-packed)
  projections: fp8 "comp3" — x = xh + xl, 64*W = Wh + Wl (all e4m3);
    acc = Wh.xh + Wh.xl + Wl.xh   (9 DoubleRow matmuls per 128x512 out tile,
    2 contraction k-tiles per matmul, 0.5 cyc/row => 3x the fp32r rate)
  qT holds 64*mw0/sqrt(64)*q, kT holds 64*k  (bf16 in SBUF)
  scT[j,i] = kT_h^T qT_h = 4096 * sc_true               (PE, bf16)
  eT = exp(scT * 1/4096) ; eT *= aexp[h]                (ACT exp w/ scale; DVE/Pool mul)
  v built with a ones-column per head (65-wide heads; v scaled back by 1/64
  on the ACT-engine psum evacuation);
  [ctx | rowsum] = eT^T @ [v | 1]                       (PE, one chain per it,
    4 it-chains batched in one PSUM tile per (h,b))
  out[i, PERM[h]*64:] = ctx * (1/rowsum)                (DVE recip + DVE bcast mul)
  out staged bf16 in true-head column order; stored in 4-head (512B) chunks.

ALiBi banding: (jt,it) 128x128 tile pairs with exp(-slope*dist) below ~3e-5
relative contribution are skipped (scores, exp, mul, pv, and table storage:
the aexp table is host-packed to only the kept spans).
"""

from contextlib import ExitStack

import numpy as np

import concourse.bass as bass
import concourse.mybir as mybir
import concourse.tile as tile

F32 = mybir.dt.float32
BF16 = mybir.dt.bfloat16
F8 = mybir.dt.float8e4

H, S, D, DH = 12, 512, 768, 64
BPC = 2                # batches per core
T = BPC * S            # tokens per core
KT = D // 128          # contraction tiles over model dim
MT = T // 128          # token tiles per core
JT = S // 128          # key tiles per sequence
VW = DH + 1            # per-head v width incl. ones column
BAND_MARGIN = 10.5

# combined fp8 input layout: columns of xw8 [D, XW_COLS]
#   X region:  xh | xl                      (2T cols)
#   QK region: per gi: wq_h | wq_l | wk_h | wk_l   (6 x 512 cols)
#   V region:  wv_h | wv_l                  (2D cols)
XW_XH = 0
XW_XL = T
XW_QK = 2 * T
XW_V = XW_QK + KT * 512
XW_COLS = XW_V + 2 * D


def _get_slopes(n):
    import math

    def pow2(n):
        start = 2 ** (-(2 ** (-(math.log2(n) - 3))))
        return [start * start**i for i in range(n)]

    if math.log2(n).is_integer():
        return pow2(n)
    cp2 = 2 ** math.floor(math.log2(n))
    return pow2(cp2) + _get_slopes(2 * cp2)[0::2][: n - cp2]


SLOPES = np.asarray(_get_slopes(H), np.float64)


def _band_dt(band_margin: float) -> list[int]:
    """Max |jt-it| (inclusive) per head; JT-1 means no banding."""
    out = []
    for sl in SLOPES:
        L = int(np.ceil(band_margin / sl))
        out.append(min((L + 127) // 128, JT - 1))
    return out


BAND = _band_dt(BAND_MARGIN)

# slot -> original head. Pairs (2gi, 2gi+1) mix a wide-band (costly exp) head
# with a narrow one so the Activation engine's load is even across the
# kernel; true-head quads (0-3, 4-7, 8-11) complete early so output stores
# can go out in contiguous 512B chunks.
PERM = [0, 4, 11, 5, 1, 6, 2, 7, 3, 10, 8, 9]
# true-head quad q covers out cols [256q, 256q+256); SLOT_OF[h] = slot of h
SLOT_OF = [PERM.index(h) for h in range(H)]
QUAD_DONE = [max(SLOT_OF[4 * q : 4 * q + 4]) for q in range(3)]  # slot finishing quad


def _spans(s):
    """Per jt: (i0, iw) kept tile-rounded column range for head slot s."""
    dt_h = BAND[PERM[s]]
    out = []
    for jt in range(JT):
        lo = max(0, jt - dt_h)
        hi = min(JT - 1, jt + dt_h)
        out.append((lo * 128, (hi - lo + 1) * 128))
    return out


def _exact_spans(s):
    """Per jt: (e0, e1) columns where exp(-slope*dist) is non-negligible.
    Score matmuls only write these; the rest of the tile-rounded span is
    memset in eT and never multiplied, so the table only stores the exact
    spans."""
    L = int(np.ceil(BAND_MARGIN / SLOPES[PERM[s]]))
    out = []
    for jt, (i0, iw) in enumerate(_spans(s)):
        e0 = max(i0, jt * 128 - L)
        e1 = min(i0 + iw, (jt + 1) * 128 + L)
        out.append((e0, e1))
    return out


# packed aexp column offsets: per (slot, jt) block of exact-span width
AE_OFF = {}
_c = 0
for _s in range(H):
    for _jt, (_e0, _e1) in enumerate(_exact_spans(_s)):
        AE_OFF[(_s, _jt)] = _c
        _c += _e1 - _e0
AE_COLS = _c


def _patch_tile_drain():
    """This walrus build rejects >1 sync-wait on one instruction; split the
    TileContext tail-drain's waits across single-wait drains."""
    from concourse.vector_clock import ScopedClock

    def _drain_and_barrier(self, tick_clock, wait_clock):
        nc = self.nc
        drain_inst = nc.sync.drain()
        wait_clock.add_sem_waits(
            drain_inst.ins, ScopedClock({None: tick_clock.global_clock})
        )
        waits = list(drain_inst.ins.sync_info.on_wait)
        if len(waits) > 1:
            drain_inst.ins.sync_info.on_wait = waits[:1]
            for w in waits[1:]:
                extra = nc.sync.drain()
                extra.ins.sync_info = mybir.SyncInfo(on_wait=[w], on_update=[])
        nc.all_engine_barrier()
        assert self.sems is not None
        popped = nc._tile_sem_poison_stack.pop()
        assert popped is self._sem_poison
        nc.clear_and_free_semaphores(list(self.sems.allocated().values()))
        nc.all_engine_barrier()

    tile.TileContext._drain_and_barrier = _drain_and_barrier


_patch_tile_drain()


def _split_multi_waits(nc):
    """This walrus build accepts at most one sync-wait per instruction; hoist
    extra waits onto single-wait NOPs emitted just before, on the same engine."""
    for fn in nc.m.functions:
        for bb in fn.blocks:
            out = []
            changed = False
            for ins in bb.instructions:
                si = ins.sync_info
                if si is not None and si.on_wait and len(si.on_wait) > 1:
                    waits = list(si.on_wait)
                    for i, w in enumerate(waits[:-1]):
                        nop = mybir.InstNoOp(
                            name=f"{ins.name}_w{i}",
                            engine=ins.engine,
                            sync_info=mybir.SyncInfo(on_wait=[w], on_update=[]),
                            bass_nofuse=True,
                        )
                        nc.register_instruction(nop, overwrite=True)
                        out.append(nop)
                    si.on_wait = waits[-1:]
                    changed = True
                out.append(ins)
            if changed:
                bb.instructions = out


def build_nc() -> bass.Bass:
    nc = bass.Bass("TRN2")
    xw8 = nc.dram_tensor("xw8", [D, XW_COLS], F8, kind="ExternalInput").ap()
    aexpP = nc.dram_tensor("aexpP", [128, AE_COLS], BF16, kind="ExternalInput").ap()
    out = nc.dram_tensor("out", [T, D], BF16, kind="ExternalOutput").ap()
    outR = out.rearrange("(mt p) d -> p mt d", p=128)

    DRM = mybir.MatmulPerfMode.DoubleRow

    with tile.TileContext(nc) as tc, ExitStack() as ctx:
        pers = ctx.enter_context(tc.tile_pool(name="pers", bufs=1))
        exp_p = ctx.enter_context(tc.tile_pool(name="exp_p", bufs=22))
        r_p = ctx.enter_context(tc.tile_pool(name="r_p", bufs=4))
        psA = ctx.enter_context(tc.tile_pool(name="psA", bufs=2, space="PSUM"))
        psS = ctx.enter_context(tc.tile_pool(name="psS", bufs=4, space="PSUM"))
        psC = ctx.enter_context(tc.tile_pool(name="psC", bufs=2, space="PSUM"))

        xw_sb = pers.tile([128, KT, XW_COLS], F8, tag="xw")
        ae_sb = pers.tile([128, AE_COLS], BF16, tag="ae")
        qT_sb = pers.tile([128, KT, T], BF16, tag="qT")
        kT_sb = pers.tile([128, KT, T], BF16, tag="kT")
        v_sb = pers.tile([128, MT, H * VW], BF16, tag="v")
        out_sb = pers.tile([128, MT, D], BF16, tag="outsb")

        def xh(g, cols):
            return xw_sb[:, 2 * g : 2 * g + 2, XW_XH + cols[0] : XW_XH + cols[1]]

        def xl(g, cols):
            return xw_sb[:, 2 * g : 2 * g + 2, XW_XL + cols[0] : XW_XL + cols[1]]

        def wqk(wi, hi_lo, gi, g):
            c0 = XW_QK + gi * 512 + (2 * wi + hi_lo) * 128
            return xw_sb[:, 2 * g : 2 * g + 2, c0 : c0 + 128]

        def wv(hi_lo, g, cols):
            c0 = XW_V + hi_lo * D
            return xw_sb[:, 2 * g : 2 * g + 2, c0 + cols[0] : c0 + cols[1]]

        # ones columns of v: cx[:, DH] accumulates the softmax denominator
        ones_cols = v_sb.rearrange("p m (h w) -> p m h w", w=VW)[:, :, :, DH : DH + 1]
        nc.gpsimd.memset(ones_cols, 1.0)

        # ---- input DMAs, ordered by first use ----
        xw8R = xw8.rearrange("(kt p) c -> p kt c", p=128)

        def ae_load(s, n=1):
            c0 = AE_OFF[(s, 0)]
            c1 = AE_OFF[(s + n, 0)] if s + n < H else AE_COLS
            nc.sync.dma_start(out=ae_sb[:, c0:c1], in_=aexpP[:, c0:c1])

        def qk_load(gi, k0=0, k1=KT):
            c0 = XW_QK + gi * 512
            nc.sync.dma_start(
                out=xw_sb[:, k0:k1, c0 : c0 + 512],
                in_=xw8R[:, k0:k1, c0 : c0 + 512],
            )

        def x_load(kt):
            nc.sync.dma_start(
                out=xw_sb[:, kt, 0 : 2 * T],
                in_=xw8[kt * 128 : (kt + 1) * 128, 0 : 2 * T],
            )

        # stagger x and the first w-column loads so the first projection
        # chain's g=0 matmuls can start ~4us in instead of waiting for all
        x_load(0)
        x_load(1)
        qk_load(0, 0, 2)
        x_load(2)
        x_load(3)
        qk_load(0, 2, 4)
        x_load(4)
        x_load(5)
        qk_load(0, 4, 6)
        ae_load(0)
        ae_load(1)
        for hi_lo in (0, 1):
            c0 = XW_V + hi_lo * D
            nc.sync.dma_start(
                out=xw_sb[:, :, c0 : c0 + D], in_=xw8R[:, :, c0 : c0 + D]
            )
        qk_load(1)
        qk_load(2)
        ae_load(2)
        qk_load(3)
        ae_load(3)
        qk_load(4)
        ae_load(4)
        qk_load(5)
        ae_load(5)
        ae_load(6)
        ae_load(7)
        ae_load(8, 4)

        TERMS = ((0, xh), (0, xl), (1, xh))  # Wh.xh + Wh.xl + Wl.xh

        def proj_qk(gi):
            for wi, dst in ((0, qT_sb), (1, kT_sb)):
                for nt in range(T // 512):
                    ps = psA.tile([128, 512], F32, tag="psA", name=f"psA_{wi}{gi}{nt}")
                    n = 0
                    for g in range(KT // 2):
                        for hi_lo, xf in TERMS:
                            nc.tensor.matmul(
                                ps,
                                lhsT=wqk(wi, hi_lo, gi, g),
                                rhs=xf(g, (nt * 512, (nt + 1) * 512)),
                                start=(n == 0),
                                stop=(n == 8),
                                perf_mode=DRM,
                            )
                            n += 1
                    nc.vector.tensor_copy(
                        out=dst[:, gi, nt * 512 : (nt + 1) * 512], in_=ps
                    )

        def proj_v(mt):
            for half, (n0, nw, h0, nh) in enumerate(
                ((0, 512, 0, 8), (512, 256, 8, 4))
            ):
                ps = psA.tile([128, 512], F32, tag="psA", name=f"psA_v{mt}{half}")
                n = 0
                for g in range(KT // 2):
                    for hi_lo, xf in TERMS:
                        nc.tensor.matmul(
                            ps[:, :nw],
                            lhsT=xf(g, (mt * 128, (mt + 1) * 128)),
                            rhs=wv(hi_lo, g, (n0, n0 + nw)),
                            start=(n == 0),
                            stop=(n == 8),
                            perf_mode=DRM,
                        )
                        n += 1
                dst = v_sb[:, mt, h0 * VW : (h0 + nh) * VW]
                dst = dst.rearrange("p (h w) -> p h w", w=VW)[:, :, 0:DH]
                src = ps[:, :nw].rearrange("p (h w) -> p h w", w=DH)
                nc.vector.tensor_scalar_mul(out=dst, in0=src, scalar1=1.0 / 64.0)

        eTs = {}

        def attend_scores(s):
            po, gi = (s % 2) * 64, s // 2
            spans = _spans(s)
            exact = _exact_spans(s)
            full = BAND[PERM[s]] == JT - 1
            for pair in (0, 1):
                jts = (2 * pair, 2 * pair + 1)
                for b in range(BPC):
                    t0 = b * S
                    eT = exp_p.tile(
                        [128, 1024], BF16, tag="eT", name=f"eT{s}_{b}_{pair}"
                    )
                    # zero the in-band-but-negligible strips PV will read;
                    # exp/mul only touch the exact spans
                    for k, jt in enumerate(jts):
                        i0, iw = spans[jt]
                        e0, e1 = exact[jt]
                        if e0 > i0:
                            nc.gpsimd.memset(
                                eT[:, k * 512 : k * 512 + e0 - i0], 0.0
                            )
                        if e1 < i0 + iw:
                            nc.gpsimd.memset(
                                eT[:, k * 512 + e1 - i0 : k * 512 + iw], 0.0
                            )
                    # one [128,512] psum tile per jt: a single matmul writer
                    # per tile keeps Tile's cross-engine RAW deps sound while
                    # letting both matmuls run before the exps (pipelined)
                    scs = []
                    for k, jt in enumerate(jts):
                        i0, iw = spans[jt]
                        e0, e1 = exact[jt]
                        sck = psS.tile(
                            [128, 512], F32, tag="sc", name=f"sc{s}_{b}_{pair}_{k}"
                        )
                        nc.tensor.matmul(
                            sck[:, e0 - i0 : e1 - i0],
                            lhsT=kT_sb[
                                po : po + DH, gi, t0 + jt * 128 : t0 + (jt + 1) * 128
                            ],
                            rhs=qT_sb[po : po + DH, gi, t0 + e0 : t0 + e1],
                            start=True,
                            stop=True,
                        )
                        scs.append(sck)
                    if False:
                        # tail slots' lagged-batch muls go to the Pool
                        # engine (idle late); Pool mis-executes in-place RMW,
                        # so exp lands in a scratch tile and the mul writes
                        # eT fresh
                        ex = exp_p.tile(
                            [128, 1024], BF16, tag="eT", name=f"ex{s}_{b}_{pair}"
                        )
                        for k, jt in enumerate(jts):
                            i0, iw = spans[jt]
                            e0, e1 = exact[jt]
                            nc.scalar.activation(
                                out=ex[:, k * 512 + e0 - i0 : k * 512 + e1 - i0],
                                in_=scs[k][:, e0 - i0 : e1 - i0],
                                func=mybir.ActivationFunctionType.Exp,
                                scale=1.0 / 4096.0,
                            )
                            c0 = AE_OFF[(s, jt)]
                            nc.gpsimd.tensor_mul(
                                out=eT[:, k * 512 + e0 - i0 : k * 512 + e1 - i0],
                                in0=ex[:, k * 512 + e0 - i0 : k * 512 + e1 - i0],
                                in1=ae_sb[:, c0 : c0 + e1 - e0],
                            )
                    else:
                        for k, jt in enumerate(jts):
                            i0, iw = spans[jt]
                            e0, e1 = exact[jt]
                            nc.scalar.activation(
                                out=eT[:, k * 512 + e0 - i0 : k * 512 + e1 - i0],
                                in_=scs[k][:, e0 - i0 : e1 - i0],
                                func=mybir.ActivationFunctionType.Exp,
                                scale=1.0 / 4096.0,
                            )
                            c0 = AE_OFF[(s, jt)]
                            nc.vector.tensor_mul(
                                out=eT[:, k * 512 + e0 - i0 : k * 512 + e1 - i0],
                                in0=eT[:, k * 512 + e0 - i0 : k * 512 + e1 - i0],
                                in1=ae_sb[:, c0 : c0 + e1 - e0],
                            )
                    eTs[(s, pair, b)] = eT

        def attend_pv(s, b):
            spans = _spans(s)
            dt_h = BAND[PERM[s]]
            cx = psC.tile([128, JT * VW], F32, tag="cx", name=f"cx{s}_{b}")
            for it in range(JT):
                jts = [jt for jt in range(JT) if abs(jt - it) <= dt_h]
                for n, jt in enumerate(jts):
                    pair, k = divmod(jt, 2)
                    i0 = spans[jt][0]
                    off = k * 512 + (it * 128 - i0)
                    nc.tensor.matmul(
                        cx[:, it * VW : (it + 1) * VW],
                        lhsT=eTs[(s, pair, b)][:, off : off + 128],
                        rhs=v_sb[:, b * JT + jt, s * VW : (s + 1) * VW],
                        start=(n == 0),
                        stop=(n == len(jts) - 1),
                    )
            cx4 = cx.rearrange("p (i w) -> p i w", w=VW)
            r = r_p.tile([128, JT, 1], F32, tag="r", name=f"r{s}_{b}")
            nc.vector.reciprocal(out=r, in_=cx4[:, :, DH : DH + 1])
            c0 = PERM[s] * DH
            nc.vector.tensor_mul(
                out=out_sb[:, b * JT : (b + 1) * JT, c0 : c0 + DH],
                in0=cx4[:, :, 0:DH],
                in1=r.broadcast_to([128, JT, DH]),
            )

        def store_q(q, b, half=None):
            c0 = q * 4 * DH
            w = 4 * DH
            if half is not None:
                c0 += half * 2 * DH
                w = 2 * DH
            m0, m1 = b * JT, (b + 1) * JT
            nc.sync.dma_start(
                out=outR[:, m0:m1, c0 : c0 + w],
                in_=out_sb[:, m0:m1, c0 : c0 + w],
            )

        # warm up the PE p-state during the input-DMA window: matmuls on a
        # locally-memset tile keep the clock ramping toward 2.4 GHz so the
        # real projection chains run at full speed
        warm = pers.tile([128, 256], BF16, tag="warm")
        nc.vector.memset(warm, 0.0)
        wps = psS.tile([128, 512], F32, tag="sc", name="warmps")
        NWARM = 24
        for i in range(NWARM):
            nc.tensor.matmul(
                wps[:, 0:256],
                lhsT=warm[:, 0:128],
                rhs=warm,
                start=(i == 0),
                stop=(i == NWARM - 1),
            )

        # schedule: projections pipelined one block ahead (proj(gi+1) emitted
        # mid-block as PE filler while exp(s0)/mul latency drains), v and PV
        # staggered per batch so PE always has work while ACT catches up.
        proj_qk(0)
        attend_scores(0)
        attend_scores(1)
        for mt in range(4):
            proj_v(mt)
        attend_pv(0, 0)
        attend_pv(1, 0)
        proj_qk(1)
        attend_scores(2)
        attend_scores(3)
        for mt in range(4, 8):
            proj_v(mt)
        attend_pv(0, 1)
        attend_pv(1, 1)
        proj_qk(2)
        attend_pv(2, 0)
        attend_pv(3, 0)
        # blocks 2..5: lagged b=1 PVs fill PE while exp of fresh scores
        # drains; the four narrow tail heads' scores are pulled forward so
        # the final stretch is pure pre-buffered PV work
        attend_scores(4)
        attend_scores(5)
        attend_pv(2, 1)
        attend_pv(3, 1)
        proj_qk(3)
        attend_pv(4, 0)
        attend_pv(5, 0)
        attend_scores(6)
        attend_scores(7)
        attend_pv(4, 1)
        attend_pv(5, 1)
        proj_qk(4)
        attend_pv(6, 0)
        attend_pv(7, 0)
        store_q(1, 0)
        proj_qk(5)
        attend_scores(8)
        attend_scores(9)
        attend_pv(6, 1)
        attend_pv(7, 1)
        store_q(1, 1)
        attend_scores(10)
        attend_scores(11)
        attend_pv(8, 0)
        store_q(0, 0)
        attend_pv(9, 0)
        attend_pv(8, 1)
        store_q(0, 1)
        attend_pv(9, 1)
        attend_pv(10, 0)
        attend_pv(11, 0)
        store_q(2, 0)
        store_q(2, 1, half=1)
        attend_pv(10, 1)
        attend_pv(11, 1)
        store_q(2, 1, half=0)
    _split_multi_waits(nc)
    return nc


def host_prep(inputs: dict):
    """Returns (shared inputs dict, per-core xw8 list)."""
    import ml_dtypes

    E4 = ml_dtypes.float8_e4m3

    hs = np.ascontiguousarray(np.asarray(inputs["hidden_states"], np.float32))
    Wq = np.asarray(inputs["Wq"], np.float32)
    Wk = np.asarray(inputs["Wk"], np.float32)
    Wv = np.asarray(inputs["Wv"], np.float32)
    qfc = np.asarray(inputs["query_fc"], np.float32)
    kfc = np.asarray(inputs["key_fc"], np.float32)
    mwt = np.asarray(inputs["mixture_weight"], np.float32)[0, :, 0, 0, :]  # [H,2]

    e = np.exp(mwt - mwt.max(-1, keepdims=True))
    mw = e / e.sum(-1, keepdims=True)
    scale = np.repeat(mw[:, 0] / np.sqrt(DH), DH).astype(np.float32)

    def permute_heads(wT):  # [D_in, D_out]: reorder out-columns to slot order
        blocks = [wT[:, PERM[s] * DH : (PERM[s] + 1) * DH] for s in range(H)]
        return np.concatenate(blocks, axis=1)

    def hilo(wT):  # [D_in, D_out] -> fp8 hi, lo of 64*wT in slot order
        w64 = permute_heads(np.asarray(wT, np.float32)) * 64.0
        hi = w64.astype(E4)
        lo = (w64 - hi.astype(np.float32)).astype(E4)
        return hi, lo

    wq_h, wq_l = hilo((Wq * scale[:, None]).T)
    wk_h, wk_l = hilo(Wk.T)
    wv_h, wv_l = hilo(Wv.T)

    # packed band-restricted bias table [128, AE_COLS]
    synthT = np.einsum("hik,hjk->hji", qfc, kfc).astype(np.float32)
    pos = np.arange(S)
    absd = np.abs(pos[None, :] - pos[:, None]).astype(np.float32)
    slopes = SLOPES.astype(np.float32)
    bias = mw[:, 1][:, None, None] * synthT - slopes[:, None, None] * absd[None]
    aexp = np.exp(bias)  # [h, j, i]
    aeP = np.zeros((128, AE_COLS), np.float32)
    for s in range(H):
        for jt, (e0, e1) in enumerate(_exact_spans(s)):
            c = AE_OFF[(s, jt)]
            aeP[:, c : c + e1 - e0] = aexp[
                PERM[s], jt * 128 : (jt + 1) * 128, e0:e1
            ]
    aeP = np.ascontiguousarray(aeP.astype(ml_dtypes.bfloat16))

    shared = dict(aexpP=aeP)
    n_cores = hs.shape[0] // BPC
    xw8s = []
    for c in range(n_cores):
        xT = hs[c * BPC : (c + 1) * BPC].reshape(T, D).T  # [D, T]
        x_h = xT.astype(E4)
        x_l = (xT - x_h.astype(np.float32)).astype(E4)
        xw = np.empty((D, XW_COLS), E4)
        xw[:, XW_XH : XW_XH + T] = x_h
        xw[:, XW_XL : XW_XL + T] = x_l
        for gi in range(KT):
            c0 = XW_QK + gi * 512
            gc = slice(gi * 128, (gi + 1) * 128)
            xw[:, c0 : c0 + 128] = wq_h[:, gc]
            xw[:, c0 + 128 : c0 + 256] = wq_l[:, gc]
            xw[:, c0 + 256 : c0 + 384] = wk_h[:, gc]
            xw[:, c0 + 384 : c0 + 512] = wk_l[:, gc]
        xw[:, XW_V : XW_V + D] = wv_h
        xw[:, XW_V + D : XW_V + 2 * D] = wv_l
        xw8s.append(np.ascontiguousarray(xw))
    return shared, xw8s


# ---------------------------------------------------------------------------
# Harness entry point: full (unsharded) inputs -> full output.
# Shards batch 16 -> 8 cores x 2, runs the SPMD Bass kernel, gathers.
# ---------------------------------------------------------------------------

N_CORES = 8
_NC_CACHE: dict = {}


def kernel(**inputs) -> np.ndarray:
    shared, xw8s = host_prep(inputs)
    if "nc" not in _NC_CACHE:
        _NC_CACHE["nc"] = build_nc()
    nc = _NC_CACHE["nc"]
    in_maps = [dict(shared, xw8=xw8s[c]) for c in range(N_CORES)]
    from concourse.bass_utils import run_bass_kernel_spmd

    res = run_bass_kernel_spmd(nc, in_maps, core_ids=list(range(N_CORES)))
    outs = [
        np.asarray(res.results[c]["out"]).astype(np.float32).reshape(BPC, S, D)
        for c in range(N_CORES)
    ]
    return np.concatenate(outs, axis=0)


# revision 34
# speedup vs baseline: 1.0518x; 1.0146x over previous
"""BertSelfAttention (synthesizer mixture + symmetric ALiBi) Bass kernel for TRN2.

Data-parallel over batch: 8 cores x 2 batches each. One SPMD program.

Decomposition (per core, batches b=0,1; heads h=0..11):
  mw = softmax(mixture_weight)                          (host, 24 floats)
  aexp[h,j,i] = exp(mw1_h*synth_h[i,j] - slope_h*|i-j|) (host table, band-packed)
  projections: fp8 "comp3" — x = xh + xl, 64*W = Wh + Wl (all e4m3);
    acc = Wh.xh + Wh.xl + Wl.xh   (9 DoubleRow matmuls per 128x512 out tile,
    2 contraction k-tiles per matmul, 0.5 cyc/row => 3x the fp32r rate)
  qT holds 64*mw0/sqrt(64)*q, kT holds 64*k  (bf16 in SBUF)
  scT[j,i] = kT_h^T qT_h = 4096 * sc_true               (PE, bf16)
  eT = exp(scT * 1/4096) ; eT *= aexp[h]                (ACT exp w/ scale; DVE/Pool mul)
  v built with a ones-column per head (65-wide heads; v scaled back by 1/64
  on the ACT-engine psum evacuation);
  [ctx | rowsum] = eT^T @ [v | 1]                       (PE, one chain per it,
    4 it-chains batched in one PSUM tile per (h,b))
  out[i, PERM[h]*64:] = ctx * (1/rowsum)                (DVE recip + DVE bcast mul)
  out staged bf16 in true-head column order; stored in 4-head (512B) chunks.

ALiBi banding: (jt,it) 128x128 tile pairs with exp(-slope*dist) below ~3e-5
relative contribution are skipped (scores, exp, mul, pv, and table storage:
the aexp table is host-packed to only the kept spans).
"""

from contextlib import ExitStack

import numpy as np

import concourse.bass as bass
import concourse.mybir as mybir
import concourse.tile as tile

F32 = mybir.dt.float32
BF16 = mybir.dt.bfloat16
F8 = mybir.dt.float8e4

H, S, D, DH = 12, 512, 768, 64
BPC = 2                # batches per core
T = BPC * S            # tokens per core
KT = D // 128          # contraction tiles over model dim
MT = T // 128          # token tiles per core
JT = S // 128          # key tiles per sequence
VW = DH + 1            # per-head v width incl. ones column
BAND_MARGIN = 10.5

# combined fp8 input layout: columns of xw8 [D, XW_COLS]
#   X region:  xh | xl                      (2T cols)
#   QK region: per gi: wq_h | wq_l | wk_h | wk_l   (6 x 512 cols)
#   V region:  wv_h | wv_l                  (2D cols)
XW_XH = 0
XW_XL = T
XW_QK = 2 * T
XW_V = XW_QK + KT * 512
XW_COLS = XW_V + 2 * D


def _get_slopes(n):
    import math

    def pow2(n):
        start = 2 ** (-(2 ** (-(math.log2(n) - 3))))
        return [start * start**i for i in range(n)]

    if math.log2(n).is_integer():
        return pow2(n)
    cp2 = 2 ** math.floor(math.log2(n))
    return pow2(cp2) + _get_slopes(2 * cp2)[0::2][: n - cp2]


SLOPES = np.asarray(_get_slopes(H), np.float64)


def _band_dt(band_margin: float) -> list[int]:
    """Max |jt-it| (inclusive) per head; JT-1 means no banding."""
    out = []
    for sl in SLOPES:
        L = int(np.ceil(band_margin / sl))
        out.append(min((L + 127) // 128, JT - 1))
    return out


BAND = _band_dt(BAND_MARGIN)

# slot -> original head. Pairs (2gi, 2gi+1) mix a wide-band (costly exp) head
# with a narrow one so the Activation engine's load is even across the
# kernel; true-head quads (0-3, 4-7, 8-11) complete early so output stores
# can go out in contiguous 512B chunks.
PERM = [0, 4, 11, 5, 1, 6, 2, 7, 3, 10, 8, 9]
# true-head quad q covers out cols [256q, 256q+256); SLOT_OF[h] = slot of h
SLOT_OF = [PERM.index(h) for h in range(H)]
QUAD_DONE = [max(SLOT_OF[4 * q : 4 * q + 4]) for q in range(3)]  # slot finishing quad


def _spans(s):
    """Per jt: (i0, iw) kept tile-rounded column range for head slot s."""
    dt_h = BAND[PERM[s]]
    out = []
    for jt in range(JT):
        lo = max(0, jt - dt_h)
        hi = min(JT - 1, jt + dt_h)
        out.append((lo * 128, (hi - lo + 1) * 128))
    return out


def _exact_spans(s):
    """Per jt: (e0, e1) columns where exp(-slope*dist) is non-negligible.
    Score matmuls only write these; the rest of the tile-rounded span is
    memset in eT and never multiplied, so the table only stores the exact
    spans."""
    L = int(np.ceil(BAND_MARGIN / SLOPES[PERM[s]]))
    out = []
    for jt, (i0, iw) in enumerate(_spans(s)):
        e0 = max(i0, jt * 128 - L)
        e1 = min(i0 + iw, (jt + 1) * 128 + L)
        out.append((e0, e1))
    return out


# packed aexp column offsets: per (slot, jt) block of exact-span width
AE_OFF = {}
_c = 0
for _s in range(H):
    for _jt, (_e0, _e1) in enumerate(_exact_spans(_s)):
        AE_OFF[(_s, _jt)] = _c
        _c += _e1 - _e0
AE_COLS = _c


def _patch_tile_drain():
    """This walrus build rejects >1 sync-wait on one instruction; split the
    TileContext tail-drain's waits across single-wait drains."""
    from concourse.vector_clock import ScopedClock

    def _drain_and_barrier(self, tick_clock, wait_clock):
        nc = self.nc
        drain_inst = nc.sync.drain()
        wait_clock.add_sem_waits(
            drain_inst.ins, ScopedClock({None: tick_clock.global_clock})
        )
        waits = list(drain_inst.ins.sync_info.on_wait)
        if len(waits) > 1:
            drain_inst.ins.sync_info.on_wait = waits[:1]
            for w in waits[1:]:
                extra = nc.sync.drain()
                extra.ins.sync_info = mybir.SyncInfo(on_wait=[w], on_update=[])
        nc.all_engine_barrier()
        assert self.sems is not None
        popped = nc._tile_sem_poison_stack.pop()
        assert popped is self._sem_poison
        nc.clear_and_free_semaphores(list(self.sems.allocated().values()))
        nc.all_engine_barrier()

    tile.TileContext._drain_and_barrier = _drain_and_barrier


_patch_tile_drain()


def _split_multi_waits(nc):
    """This walrus build accepts at most one sync-wait per instruction; hoist
    extra waits onto single-wait NOPs emitted just before, on the same engine."""
    for fn in nc.m.functions:
        for bb in fn.blocks:
            out = []
            changed = False
            for ins in bb.instructions:
                si = ins.sync_info
                if si is not None and si.on_wait and len(si.on_wait) > 1:
                    waits = list(si.on_wait)
                    for i, w in enumerate(waits[:-1]):
                        nop = mybir.InstNoOp(
                            name=f"{ins.name}_w{i}",
                            engine=ins.engine,
                            sync_info=mybir.SyncInfo(on_wait=[w], on_update=[]),
                            bass_nofuse=True,
                        )
                        nc.register_instruction(nop, overwrite=True)
                        out.append(nop)
                    si.on_wait = waits[-1:]
                    changed = True
                out.append(ins)
            if changed:
                bb.instructions = out


def build_nc() -> bass.Bass:
    nc = bass.Bass("TRN2")
    xw8 = nc.dram_tensor("xw8", [D, XW_COLS], F8, kind="ExternalInput").ap()
    aexpP = nc.dram_tensor("aexpP", [128, AE_COLS], BF16, kind="ExternalInput").ap()
    out = nc.dram_tensor("out", [T, D], BF16, kind="ExternalOutput").ap()
    outR = out.rearrange("(mt p) d -> p mt d", p=128)

    DRM = mybir.MatmulPerfMode.DoubleRow

    with tile.TileContext(nc) as tc, ExitStack() as ctx:
        pers = ctx.enter_context(tc.tile_pool(name="pers", bufs=1))
        exp_p = ctx.enter_context(tc.tile_pool(name="exp_p", bufs=22))
        r_p = ctx.enter_context(tc.tile_pool(name="r_p", bufs=4))
        psA = ctx.enter_context(tc.tile_pool(name="psA", bufs=2, space="PSUM"))
        psS = ctx.enter_context(tc.tile_pool(name="psS", bufs=4, space="PSUM"))
        psC = ctx.enter_context(tc.tile_pool(name="psC", bufs=2, space="PSUM"))

        xw_sb = pers.tile([128, KT, XW_COLS], F8, tag="xw")
        ae_sb = pers.tile([128, AE_COLS], BF16, tag="ae")
        qT_sb = pers.tile([128, KT, T], BF16, tag="qT")
        kT_sb = pers.tile([128, KT, T], BF16, tag="kT")
        v_sb = pers.tile([128, MT, H * VW], BF16, tag="v")
        out_sb = pers.tile([128, MT, D], BF16, tag="outsb")

        def xh(g, cols):
            return xw_sb[:, 2 * g : 2 * g + 2, XW_XH + cols[0] : XW_XH + cols[1]]

        def xl(g, cols):
            return xw_sb[:, 2 * g : 2 * g + 2, XW_XL + cols[0] : XW_XL + cols[1]]

        def wqk(wi, hi_lo, gi, g):
            c0 = XW_QK + gi * 512 + (2 * wi + hi_lo) * 128
            return xw_sb[:, 2 * g : 2 * g + 2, c0 : c0 + 128]

        def wv(hi_lo, g, cols):
            c0 = XW_V + hi_lo * D
            return xw_sb[:, 2 * g : 2 * g + 2, c0 + cols[0] : c0 + cols[1]]

        # ones columns of v: cx[:, DH] accumulates the softmax denominator
        ones_cols = v_sb.rearrange("p m (h w) -> p m h w", w=VW)[:, :, :, DH : DH + 1]
        nc.gpsimd.memset(ones_cols, 1.0)

        # ---- input DMAs, ordered by first use ----
        xw8R = xw8.rearrange("(kt p) c -> p kt c", p=128)

        def ae_load(s, n=1):
            c0 = AE_OFF[(s, 0)]
            c1 = AE_OFF[(s + n, 0)] if s + n < H else AE_COLS
            nc.sync.dma_start(out=ae_sb[:, c0:c1], in_=aexpP[:, c0:c1])

        def qk_load(gi, k0=0, k1=KT):
            c0 = XW_QK + gi * 512
            nc.sync.dma_start(
                out=xw_sb[:, k0:k1, c0 : c0 + 512],
                in_=xw8R[:, k0:k1, c0 : c0 + 512],
            )

        def x_load(kt):
            nc.sync.dma_start(
                out=xw_sb[:, kt, 0 : 2 * T],
                in_=xw8[kt * 128 : (kt + 1) * 128, 0 : 2 * T],
            )

        # stagger x and the first w-column loads so the first projection
        # chain's g=0 matmuls can start ~4us in instead of waiting for all
        x_load(0)
        x_load(1)
        qk_load(0, 0, 2)
        x_load(2)
        x_load(3)
        qk_load(0, 2, 4)
        x_load(4)
        x_load(5)
        qk_load(0, 4, 6)
        for hi_lo in (0, 1):
            c0 = XW_V + hi_lo * D
            nc.sync.dma_start(
                out=xw_sb[:, :, c0 : c0 + D], in_=xw8R[:, :, c0 : c0 + D]
            )
        ae_load(0)
        ae_load(1)
        qk_load(1)
        qk_load(2)
        ae_load(2)
        qk_load(3)
        ae_load(3)
        qk_load(4)
        ae_load(4)
        qk_load(5)
        ae_load(5)
        ae_load(6)
        ae_load(7)
        ae_load(8, 4)

        TERMS = ((0, xh), (0, xl), (1, xh))  # Wh.xh + Wh.xl + Wl.xh

        def proj_qk(gi):
            for wi, dst in ((0, qT_sb), (1, kT_sb)):
                for nt in range(T // 512):
                    ps = psA.tile([128, 512], F32, tag="psA", name=f"psA_{wi}{gi}{nt}")
                    n = 0
                    for g in range(KT // 2):
                        for hi_lo, xf in TERMS:
                            nc.tensor.matmul(
                                ps,
                                lhsT=wqk(wi, hi_lo, gi, g),
                                rhs=xf(g, (nt * 512, (nt + 1) * 512)),
                                start=(n == 0),
                                stop=(n == 8),
                                perf_mode=DRM,
                            )
                            n += 1
                    nc.vector.tensor_copy(
                        out=dst[:, gi, nt * 512 : (nt + 1) * 512], in_=ps
                    )

        def proj_v(mt):
            for half, (n0, nw, h0, nh) in enumerate(
                ((0, 512, 0, 8), (512, 256, 8, 4))
            ):
                ps = psA.tile([128, 512], F32, tag="psA", name=f"psA_v{mt}{half}")
                n = 0
                for g in range(KT // 2):
                    for hi_lo, xf in TERMS:
                        nc.tensor.matmul(
                            ps[:, :nw],
                            lhsT=xf(g, (mt * 128, (mt + 1) * 128)),
                            rhs=wv(hi_lo, g, (n0, n0 + nw)),
                            start=(n == 0),
                            stop=(n == 8),
                            perf_mode=DRM,
                        )
                        n += 1
                dst = v_sb[:, mt, h0 * VW : (h0 + nh) * VW]
                dst = dst.rearrange("p (h w) -> p h w", w=VW)[:, :, 0:DH]
                src = ps[:, :nw].rearrange("p (h w) -> p h w", w=DH)
                nc.vector.tensor_scalar_mul(out=dst, in0=src, scalar1=1.0 / 64.0)

        eTs = {}

        def attend_scores(s):
            po, gi = (s % 2) * 64, s // 2
            spans = _spans(s)
            exact = _exact_spans(s)
            full = BAND[PERM[s]] == JT - 1
            for pair in (0, 1):
                jts = (2 * pair, 2 * pair + 1)
                for b in range(BPC):
                    t0 = b * S
                    eT = exp_p.tile(
                        [128, 1024], BF16, tag="eT", name=f"eT{s}_{b}_{pair}"
                    )
                    # zero the in-band-but-negligible strips PV will read;
                    # exp/mul only touch the exact spans
                    for k, jt in enumerate(jts):
                        i0, iw = spans[jt]
                        e0, e1 = exact[jt]
                        if e0 > i0:
                            nc.gpsimd.memset(
                                eT[:, k * 512 : k * 512 + e0 - i0], 0.0
                            )
                        if e1 < i0 + iw:
                            nc.gpsimd.memset(
                                eT[:, k * 512 + e1 - i0 : k * 512 + iw], 0.0
                            )
                    # one [128,512] psum tile per jt: a single matmul writer
                    # per tile keeps Tile's cross-engine RAW deps sound while
                    # letting both matmuls run before the exps (pipelined)
                    scs = []
                    for k, jt in enumerate(jts):
                        i0, iw = spans[jt]
                        e0, e1 = exact[jt]
                        sck = psS.tile(
                            [128, 512], F32, tag="sc", name=f"sc{s}_{b}_{pair}_{k}"
                        )
                        nc.tensor.matmul(
                            sck[:, e0 - i0 : e1 - i0],
                            lhsT=kT_sb[
                                po : po + DH, gi, t0 + jt * 128 : t0 + (jt + 1) * 128
                            ],
                            rhs=qT_sb[po : po + DH, gi, t0 + e0 : t0 + e1],
                            start=True,
                            stop=True,
                        )
                        scs.append(sck)
                    if False:
                        # tail slots' lagged-batch muls go to the Pool
                        # engine (idle late); Pool mis-executes in-place RMW,
                        # so exp lands in a scratch tile and the mul writes
                        # eT fresh
                        ex = exp_p.tile(
                            [128, 1024], BF16, tag="eT", name=f"ex{s}_{b}_{pair}"
                        )
                        for k, jt in enumerate(jts):
                            i0, iw = spans[jt]
                            e0, e1 = exact[jt]
                            nc.scalar.activation(
                                out=ex[:, k * 512 + e0 - i0 : k * 512 + e1 - i0],
                                in_=scs[k][:, e0 - i0 : e1 - i0],
                                func=mybir.ActivationFunctionType.Exp,
                                scale=1.0 / 4096.0,
                            )
                            c0 = AE_OFF[(s, jt)]
                            nc.gpsimd.tensor_mul(
                                out=eT[:, k * 512 + e0 - i0 : k * 512 + e1 - i0],
                                in0=ex[:, k * 512 + e0 - i0 : k * 512 + e1 - i0],
                                in1=ae_sb[:, c0 : c0 + e1 - e0],
                            )
                    else:
                        for k, jt in enumerate(jts):
                            i0, iw = spans[jt]
                            e0, e1 = exact[jt]
                            nc.scalar.activation(
                                out=eT[:, k * 512 + e0 - i0 : k * 512 + e1 - i0],
                                in_=scs[k][:, e0 - i0 : e1 - i0],
                                func=mybir.ActivationFunctionType.Exp,
                                scale=1.0 / 4096.0,
                            )
                            c0 = AE_OFF[(s, jt)]
                            nc.vector.tensor_mul(
                                out=eT[:, k * 512 + e0 - i0 : k * 512 + e1 - i0],
                                in0=eT[:, k * 512 + e0 - i0 : k * 512 + e1 - i0],
                                in1=ae_sb[:, c0 : c0 + e1 - e0],
                            )
                    eTs[(s, pair, b)] = eT

        def attend_pv(s, b):
            spans = _spans(s)
            dt_h = BAND[PERM[s]]
            cx = psC.tile([128, JT * VW], F32, tag="cx", name=f"cx{s}_{b}")
            for it in range(JT):
                jts = [jt for jt in range(JT) if abs(jt - it) <= dt_h]
                for n, jt in enumerate(jts):
                    pair, k = divmod(jt, 2)
                    i0 = spans[jt][0]
                    off = k * 512 + (it * 128 - i0)
                    nc.tensor.matmul(
                        cx[:, it * VW : (it + 1) * VW],
                        lhsT=eTs[(s, pair, b)][:, off : off + 128],
                        rhs=v_sb[:, b * JT + jt, s * VW : (s + 1) * VW],
                        start=(n == 0),
                        stop=(n == len(jts) - 1),
                    )
            cx4 = cx.rearrange("p (i w) -> p i w", w=VW)
            r = r_p.tile([128, JT, 1], F32, tag="r", name=f"r{s}_{b}")
            nc.vector.reciprocal(out=r, in_=cx4[:, :, DH : DH + 1])
            c0 = PERM[s] * DH
            nc.vector.tensor_mul(
                out=out_sb[:, b * JT : (b + 1) * JT, c0 : c0 + DH],
                in0=cx4[:, :, 0:DH],
                in1=r.broadcast_to([128, JT, DH]),
            )

        def store_q(q, b, half=None):
            c0 = q * 4 * DH
            w = 4 * DH
            if half is not None:
                c0 += half * 2 * DH
                w = 2 * DH
            m0, m1 = b * JT, (b + 1) * JT
            nc.sync.dma_start(
                out=outR[:, m0:m1, c0 : c0 + w],
                in_=out_sb[:, m0:m1, c0 : c0 + w],
            )

        # warm up the PE p-state during the input-DMA window: matmuls on a
        # locally-memset tile keep the clock ramping toward 2.4 GHz so the
        # real projection chains run at full speed
        warm = pers.tile([128, 256], BF16, tag="warm")
        nc.vector.memset(warm, 0.0)
        wps = psS.tile([128, 512], F32, tag="sc", name="warmps")
        NWARM = 24
        for i in range(NWARM):
            nc.tensor.matmul(
                wps[:, 0:256],
                lhsT=warm[:, 0:128],
                rhs=warm,
                start=(i == 0),
                stop=(i == NWARM - 1),
            )

        # schedule: projections pipelined one block ahead (proj(gi+1) emitted
        # mid-block as PE filler while exp(s0)/mul latency drains), v and PV
        # staggered per batch so PE always has work while ACT catches up.
        proj_qk(0)
        attend_scores(0)
        attend_scores(1)
        for mt in range(4):
            proj_v(mt)
        attend_pv(0, 0)
        attend_pv(1, 0)
        proj_qk(1)
        attend_scores(2)
        attend_scores(3)
        for mt in range(4, 8):
            proj_v(mt)
        attend_pv(0, 1)
        attend_pv(1, 1)
        proj_qk(2)
        attend_pv(2, 0)
        attend_pv(3, 0)
        # blocks 2..5: lagged b=1 PVs fill PE while exp of fresh scores
        # drains; the four narrow tail heads' scores are pulled forward so
        # the final stretch is pure pre-buffered PV work
        attend_scores(4)
        attend_scores(5)
        attend_pv(2, 1)
        attend_pv(3, 1)
        proj_qk(3)
        attend_pv(4, 0)
        attend_pv(5, 0)
        attend_scores(6)
        attend_scores(7)
        attend_pv(4, 1)
        attend_pv(5, 1)
        proj_qk(4)
        attend_pv(6, 0)
        attend_pv(7, 0)
        store_q(1, 0)
        proj_qk(5)
        attend_scores(8)
        attend_scores(9)
        attend_pv(6, 1)
        attend_pv(7, 1)
        store_q(1, 1)
        attend_scores(10)
        attend_scores(11)
        attend_pv(8, 0)
        store_q(0, 0)
        attend_pv(9, 0)
        attend_pv(8, 1)
        store_q(0, 1)
        attend_pv(9, 1)
        attend_pv(10, 0)
        attend_pv(11, 0)
        store_q(2, 0)
        store_q(2, 1, half=1)
        attend_pv(10, 1)
        attend_pv(11, 1)
        store_q(2, 1, half=0)
    _split_multi_waits(nc)
    return nc


def host_prep(inputs: dict):
    """Returns (shared inputs dict, per-core xw8 list)."""
    import ml_dtypes

    E4 = ml_dtypes.float8_e4m3

    hs = np.ascontiguousarray(np.asarray(inputs["hidden_states"], np.float32))
    Wq = np.asarray(inputs["Wq"], np.float32)
    Wk = np.asarray(inputs["Wk"], np.float32)
    Wv = np.asarray(inputs["Wv"], np.float32)
    qfc = np.asarray(inputs["query_fc"], np.float32)
    kfc = np.asarray(inputs["key_fc"], np.float32)
    mwt = np.asarray(inputs["mixture_weight"], np.float32)[0, :, 0, 0, :]  # [H,2]

    e = np.exp(mwt - mwt.max(-1, keepdims=True))
    mw = e / e.sum(-1, keepdims=True)
    scale = np.repeat(mw[:, 0] / np.sqrt(DH), DH).astype(np.float32)

    def permute_heads(wT):  # [D_in, D_out]: reorder out-columns to slot order
        blocks = [wT[:, PERM[s] * DH : (PERM[s] + 1) * DH] for s in range(H)]
        return np.concatenate(blocks, axis=1)

    def hilo(wT):  # [D_in, D_out] -> fp8 hi, lo of 64*wT in slot order
        w64 = permute_heads(np.asarray(wT, np.float32)) * 64.0
        hi = w64.astype(E4)
        lo = (w64 - hi.astype(np.float32)).astype(E4)
        return hi, lo

    wq_h, wq_l = hilo((Wq * scale[:, None]).T)
    wk_h, wk_l = hilo(Wk.T)
    wv_h, wv_l = hilo(Wv.T)

    # packed band-restricted bias table [128, AE_COLS]
    synthT = np.einsum("hik,hjk->hji", qfc, kfc).astype(np.float32)
    pos = np.arange(S)
    absd = np.abs(pos[None, :] - pos[:, None]).astype(np.float32)
    slopes = SLOPES.astype(np.float32)
    bias = mw[:, 1][:, None, None] * synthT - slopes[:, None, None] * absd[None]
    aexp = np.exp(bias)  # [h, j, i]
    aeP = np.zeros((128, AE_COLS), np.float32)
    for s in range(H):
        for jt, (e0, e1) in enumerate(_exact_spans(s)):
            c = AE_OFF[(s, jt)]
            aeP[:, c : c + e1 - e0] = aexp[
                PERM[s], jt * 128 : (jt + 1) * 128, e0:e1
            ]
    aeP = np.ascontiguousarray(aeP.astype(ml_dtypes.bfloat16))

    shared = dict(aexpP=aeP)
    n_cores = hs.shape[0] // BPC
    xw8s = []
    for c in range(n_cores):
        xT = hs[c * BPC : (c + 1) * BPC].reshape(T, D).T  # [D, T]
        x_h = xT.astype(E4)
        x_l = (xT - x_h.astype(np.float32)).astype(E4)
        xw = np.empty((D, XW_COLS), E4)
        xw[:, XW_XH : XW_XH + T] = x_h
        xw[:, XW_XL : XW_XL + T] = x_l
        for gi in range(KT):
            c0 = XW_QK + gi * 512
            gc = slice(gi * 128, (gi + 1) * 128)
            xw[:, c0 : c0 + 128] = wq_h[:, gc]
            xw[:, c0 + 128 : c0 + 256] = wq_l[:, gc]
            xw[:, c0 + 256 : c0 + 384] = wk_h[:, gc]
            xw[:, c0 + 384 : c0 + 512] = wk_l[:, gc]
        xw[:, XW_V : XW_V + D] = wv_h
        xw[:, XW_V + D : XW_V + 2 * D] = wv_l
        xw8s.append(np.ascontiguousarray(xw))
    return shared, xw8s


# ---------------------------------------------------------------------------
# Harness entry point: full (unsharded) inputs -> full output.
# Shards batch 16 -> 8 cores x 2, runs the SPMD Bass kernel, gathers.
# ---------------------------------------------------------------------------

N_CORES = 8
_NC_CACHE: dict = {}


def kernel(**inputs) -> np.ndarray:
    shared, xw8s = host_prep(inputs)
    if "nc" not in _NC_CACHE:
        _NC_CACHE["nc"] = build_nc()
    nc = _NC_CACHE["nc"]
    in_maps = [dict(shared, xw8=xw8s[c]) for c in range(N_CORES)]
    from concourse.bass_utils import run_bass_kernel_spmd

    res = run_bass_kernel_spmd(nc, in_maps, core_ids=list(range(N_CORES)))
    outs = [
        np.asarray(res.results[c]["out"]).astype(np.float32).reshape(BPC, S, D)
        for c in range(N_CORES)
    ]
    return np.concatenate(outs, axis=0)
